# revision 10
# baseline (speedup 1.0000x reference)
"""CapsuleLayer Bass/Tile kernel for TRN2 (one NeuronCore; replicated SPMD x8).

Per core: xin [Bc, 2048] f32, kpad [2048, 176] f16 (kernel cols 0:160,
col 160:176 = 0.1*sum of capsule blocks), ident [128, 128] f32r.
Output yout [Bc, 16] f32.

Pipeline per 128-sample b-tile:
  DMA xin tile -> SBUF (natural layout, f32)
  PE transpose 16x [128,128] -> 2x PSUM [128,1024] (f32r pass-through)
  ACT/DVE copy PSUM -> SBUF f16 (transposed tiles = matmul lhsT)
  PE matmul (data-as-weights) x16 accumulating f16 kpad-streams
    -> PSUM hat [128, 176] f32
  copy hat[:, :160] -> f32 wide staging, [160:176] -> f32 s1 staging
Routing per group of G b-tiles on wide [128, G*160] f32 layout.
"""

from dataclasses import dataclass

import numpy as np

import concourse.bacc as bacc
import concourse.tile as tile
from concourse import mybir

NCAP = 10
DCAP = 16
EPS = 1e-7
D = 2048
NCOL = NCAP * DCAP  # 160
S1COL = NCOL + DCAP  # 176


@dataclass
class Cfg:
    n_btiles: int = 16          # 128-sample tiles per core
    group_sizes: str = "8,4,2,1,1"
    n_cores: int = 8
    reps: int = 1
    dma_btiles: int = 1         # b-tiles per input DMA (1/2/4/8)
    ablate: str = "full"        # full | noroute
    loop_reps: int = 0          # >0: wrap body in a hardware For_i loop
    stage_eng: str = "AD"       # engines for the 2 half stage copies
    hat_eng: str = "A"          # engine for hat [160] f32 copy
    s1_eng: str = "A"           # engine for s1 [16] f32 copy
    wsum_mul_eng: str = "D"     # engine for wsum big muls
    dots_mul_eng: str = "D"     # engine for dots big muls
    use_divide: bool = False    # DVE divide op in squash (no ISA support)
    hoist_kpad: bool = True     # load kpad outside the loop body
    nat_bufs: int = 6
    tt_bufs: int = 6
    pstage_bufs: int = 3
    phat_bufs: int = 2

    @property
    def bc(self):
        return self.n_btiles * 128


def make_kpad(kernel: np.ndarray) -> np.ndarray:
    """[2048, 160] f32 -> [2048, 176] bf16 with col 160:176 = 0.1 * capsule sum."""
    d, ncol = kernel.shape
    assert (d, ncol) == (D, NCOL)
    kpad = np.zeros((d, S1COL), dtype=np.float64)
    kpad[:, :NCOL] = kernel
    kpad[:, NCOL:S1COL] = 0.1 * kernel.astype(np.float64).reshape(
        d, NCAP, DCAP).sum(axis=1)
    return kpad.astype(np.float16)


def build(cfg: Cfg):
    nc = bacc.Bacc("TRN2", target_bir_lowering=False, debug=False,
                   num_devices=cfg.n_cores)
    f32 = mybir.dt.float32
    f32r = mybir.dt.float32r
    f16 = mybir.dt.float16

    NB = cfg.n_btiles
    sizes = [int(s) for s in cfg.group_sizes.split(",")]
    assert sum(sizes) == NB

    def eng(c):
        return {"A": nc.scalar, "D": nc.vector, "P": nc.gpsimd}[c]

    def copy_with(c, out, in_):
        if c == "A":
            nc.scalar.copy(out, in_)
        elif c == "D":
            nc.vector.tensor_copy(out, in_)
        else:
            nc.gpsimd.tensor_copy(out, in_)

    eps_t = nc.alloc_sbuf_tensor("const-eps", [128, 1], f32)
    nc.gpsimd.memset(eps_t.ap(), EPS)
    nc.const_aps.aps[(f32, EPS)] = eps_t.ap()
    nc.all_engine_barrier()

    xin = nc.dram_tensor("xin", [cfg.bc, D], f32r, kind="ExternalInput")
    kpad = nc.dram_tensor("kpad", [D, S1COL], f16, kind="ExternalInput")
    ident = nc.dram_tensor("ident", [128, 128], f32r, kind="ExternalInput")
    seed = nc.dram_tensor("seed", [128, DCAP], f32, kind="ExternalInput")
    yout = nc.dram_tensor("yout", [cfg.bc, DCAP], f32, kind="ExternalOutput")

    with tile.TileContext(nc) as tc:
        with (
            tc.tile_pool(name="const", bufs=1) as constp,
            tc.tile_pool(name="nat", bufs=cfg.nat_bufs) as natp,
            tc.tile_pool(name="tT", bufs=cfg.tt_bufs) as tTp,
            tc.tile_pool(name="pstage", bufs=cfg.pstage_bufs, space="PSUM") as pstagep,
            tc.tile_pool(name="phat", bufs=cfg.phat_bufs, space="PSUM") as phatp,
            tc.tile_pool(name="hatw", bufs=2) as hatwp,
            tc.tile_pool(name="rt", bufs=2) as rtp,
            tc.tile_pool(name="sm", bufs=2) as smp,
            tc.tile_pool(name="outs", bufs=2) as outsp,
        ):
            id_t = constp.tile([128, 128], f32r, tag="ident")
            nc.sync.dma_start(id_t[:], ident[:, :])
            seed_t = constp.tile([128, DCAP], f32, tag="seed")
            nc.sync.dma_start(seed_t[:], seed[:, :])
            kp_t = constp.tile([128, 16 * S1COL], f16, tag="kpad")

            def load_kpad():
                # kpad[j*128 + p, c] -> kp_t[p, j*176 + c]
                nc.sync.dma_start(
                    kp_t[:].rearrange("p (j c) -> p j c", j=16),
                    kpad[:, :].rearrange("(j p) c -> p j c", p=128),
                )

            xv = xin[:, :].rearrange("(t p) d -> t p d", p=128)

            DB = cfg.dma_btiles
            nat_slices = {}  # i -> (tile, col offset)
            kpad_loaded = [False]

            def load_chunk(i0):
                nat = natp.tile([128, DB * D], f32r, tag="nat")
                if DB == 1:
                    nc.sync.dma_start(nat[:], xv[i0])
                else:
                    nc.sync.dma_start(
                        nat[:].rearrange("p (t d) -> p t d", t=DB),
                        xin[:, :].rearrange("(c t p) d -> c p t d",
                                            t=DB, p=128)[i0 // DB],
                    )
                for t in range(DB):
                    nat_slices[i0 + t] = (nat, t * D)
                if not kpad_loaded[0] and not cfg.hoist_kpad:
                    kpad_loaded[0] = True
                    load_kpad()

            def run_group(i0, G, gi):
                yv = yout[i0 * 128:(i0 + G) * 128, :].rearrange(
                    "(g p) d -> p g d", p=128)
                hatw = hatwp.tile([128, G * NCOL], f32, tag="hatw")
                s1w = hatwp.tile([128, G * DCAP], f32, tag="s1w")
                for g in range(G):
                    i = i0 + g
                    if i % DB == 0:
                        load_chunk(i)
                    nat, off = nat_slices.pop(i)
                    tts = []
                    for h in range(2):
                        ps = pstagep.tile([128, 1024], f32r, tag="pstage")
                        for jj in range(8):
                            j = h * 8 + jj
                            nc.tensor.matmul(
                                ps[:, jj * 128:(jj + 1) * 128],
                                nat[:, off + j * 128:off + (j + 1) * 128],
                                id_t[:],
                                is_transpose=True,
                                start=(jj % 4 == 0),
                                stop=(jj % 4 == 3),
                            )
                        tt = tTp.tile([128, 1024], f16, tag="tT")
                        copy_with(cfg.stage_eng[h], tt[:], ps[:])
                        tts.append(tt)
                    ph = phatp.tile([128, S1COL], f32, tag="phat")
                    for j in range(16):
                        h, jj = divmod(j, 8)
                        nc.tensor.matmul(
                            ph[:],
                            tts[h][:, jj * 128:(jj + 1) * 128],
                            kp_t[:, j * S1COL:(j + 1) * S1COL],
                            start=(j == 0),
                            stop=(j == 15),
                        )
                    copy_with(cfg.hat_eng,
                              hatw[:, g * NCOL:(g + 1) * NCOL], ph[:, :NCOL])
                    copy_with(cfg.s1_eng,
                              s1w[:, g * DCAP:(g + 1) * DCAP],
                              ph[:, NCOL:S1COL])
                if cfg.ablate == "noroute":
                    nc.sync.dma_start(
                        yv,
                        s1w[:].rearrange("p (g q) -> p g q", g=G))
                    return

                # ---- routing on [128, G*160] bf16 ----
                H = hatw[:]
                Hgnd = H.rearrange("p (g n d) -> p g n d", g=G, n=NCAP)
                wse = eng(cfg.wsum_mul_eng)
                dse = eng(cfg.dots_mul_eng)

                def squash_comb(su, r, tag):
                    """combined scale c s.t. v = c * su, where s = su * r
                    (r None -> s = su). Returns [128, G] AP."""
                    sq = smp.tile([128, G * DCAP], f32, tag=f"sq{tag}")
                    nc.vector.tensor_mul(sq[:], su, su)
                    m2 = smp.tile([128, G], f32, tag=f"m2{tag}")
                    nc.vector.tensor_reduce(
                        m2[:], sq[:].rearrange("p (g d) -> p g d", g=G),
                        axis=mybir.AxisListType.X, op=mybir.AluOpType.add)
                    if r is not None:
                        rr = smp.tile([128, G], f32, tag=f"rr{tag}")
                        nc.vector.tensor_mul(rr[:], r, r)
                        n2 = smp.tile([128, G], f32, tag=f"n2{tag}")
                        nc.vector.tensor_mul(n2[:], m2[:], rr[:])
                    else:
                        n2 = m2
                    sr = smp.tile([128, G], f32, tag=f"sr{tag}")
                    nc.scalar.activation(sr[:], n2[:],
                                         mybir.ActivationFunctionType.Sqrt,
                                         bias=EPS)
                    den = smp.tile([128, G], f32, tag=f"den{tag}")
                    nc.vector.scalar_tensor_tensor(
                        den[:], n2[:], 1.0, sr[:],
                        op0=mybir.AluOpType.add, op1=mybir.AluOpType.mult)
                    sc = smp.tile([128, G], f32, tag=f"sc{tag}")
                    if cfg.use_divide:
                        nc.vector.tensor_tensor(
                            sc[:], n2[:], den[:], op=mybir.AluOpType.divide)
                    else:
                        rec = smp.tile([128, G], f32, tag=f"rec{tag}")
                        nc.vector.reciprocal(rec[:], den[:])
                        nc.vector.tensor_mul(sc[:], n2[:], rec[:])
                    if r is not None:
                        comb = smp.tile([128, G], f32, tag=f"comb{tag}")
                        nc.vector.tensor_mul(comb[:], sc[:], r)
                        return comb
                    return sc

                def dots_d(src_gd, tag):
                    """r[g,n] = sum_d H[g,n,d] * src[g,d] -> [128, G*NCAP] f32"""
                    tmp = rtp.tile([128, G * NCOL], f32, tag=f"dt{tag}")
                    bc = (src_gd.rearrange("p (g d) -> p g d", g=G)
                          .unsqueeze(2).broadcast_to((128, G, NCAP, DCAP)))
                    dse.tensor_mul(
                        tmp[:].rearrange("p (g n d) -> p g n d", g=G, n=NCAP),
                        Hgnd, bc)
                    out = rtp.tile([128, G * NCAP], f32, tag=f"dr{tag}")
                    nc.vector.tensor_reduce(
                        out[:], tmp[:].rearrange("p (g n d) -> p g n d",
                                                 g=G, n=NCAP),
                        axis=mybir.AxisListType.X, op=mybir.AluOpType.add)
                    return out

                def wsum_n(e_gn, tag):
                    """su[g,d] = sum_n H[g,n,d] * e[g,n] -> [128, G*DCAP] f32"""
                    tmp = rtp.tile([128, G * NCOL], f32, tag=f"wt{tag}")
                    bc = e_gn.unsqueeze(3).broadcast_to((128, G, NCAP, DCAP))
                    wse.tensor_mul(
                        tmp[:].rearrange("p (g n d) -> p g n d", g=G, n=NCAP),
                        Hgnd, bc)
                    out = rtp.tile([128, G * DCAP], f32, tag=f"ws{tag}")
                    nc.vector.tensor_reduce(
                        out[:], tmp[:].rearrange("p (g n d) -> p g d n",
                                                 g=G, n=NCAP),
                        axis=mybir.AxisListType.X, op=mybir.AluOpType.add)
                    return out

                def softmax_recip(t_gn, tag):
                    """e = exp(t) [128, G*NCAP] f32; r = 1/sum_n e [128, G]"""
                    e = rtp.tile([128, G * NCAP], f32, tag=f"e{tag}")
                    nc.scalar.activation(e[:], t_gn,
                                         mybir.ActivationFunctionType.Exp)
                    se = smp.tile([128, G], f32, tag=f"se{tag}")
                    nc.vector.tensor_reduce(
                        se[:], e[:].rearrange("p (g n) -> p g n", g=G),
                        axis=mybir.AxisListType.X, op=mybir.AluOpType.add)
                    ri = smp.tile([128, G], f32, tag=f"ri{tag}")
                    nc.vector.reciprocal(ri[:], se[:])
                    return e, ri

                gv = lambda ap: ap.rearrange("p (g d) -> p g d", g=G)
                nv = lambda ap: ap.rearrange("p (g n) -> p g n", g=G)

                # iter 1: s1 (pre-scaled mean) came from the matmul
                comb1 = squash_comb(s1w[:], None, "1")  # v1 = comb1*s1
                r2 = dots_d(s1w[:], "2")                # u.s1
                t2 = rtp.tile([128, G * NCAP], f32, tag="t2")
                nc.vector.tensor_mul(
                    nv(t2[:]), nv(r2[:]),
                    comb1[:].unsqueeze(2).broadcast_to((128, G, NCAP)))

                # iter 2
                e2, r2i = softmax_recip(t2[:], "2")
                s2u = wsum_n(nv(e2[:]), "2")
                comb2 = squash_comb(s2u[:], r2i[:], "2")  # v2 = comb2*s2u
                r3 = dots_d(s2u[:], "3")                  # u.s2u
                t3 = rtp.tile([128, G * NCAP], f32, tag="t3")
                nc.vector.tensor_mul(
                    nv(t3[:]), nv(r3[:]),
                    comb2[:].unsqueeze(2).broadcast_to((128, G, NCAP)))
                nc.vector.tensor_add(t3[:], t3[:], t2[:])

                # iter 3
                e3, r3i = softmax_recip(t3[:], "3")
                s3u = wsum_n(nv(e3[:]), "3")
                comb3 = squash_comb(s3u[:], r3i[:], "3")
                v3 = outsp.tile([128, G * DCAP], f32, tag="v3")
                nc.vector.tensor_mul(
                    gv(v3[:]), gv(s3u[:]),
                    comb3[:].unsqueeze(2).broadcast_to((128, G, DCAP)))
                nc.sync.dma_start(
                    yv,
                    v3[:].rearrange("p (g d) -> p g d", g=G))

            def run_all():
                i0 = 0
                for gi, G in enumerate(sizes):
                    run_group(i0, G, gi)
                    i0 += G

            if cfg.hoist_kpad:
                load_kpad()
            if cfg.loop_reps > 0:
                with tc.For_i(0, cfg.loop_reps, 1,
                              hint_engines=(mybir.EngineType.PE,)):
                    run_all()
            else:
                for _rep in range(cfg.reps):
                    run_all()

    nc.compile()
    return nc


# ---------------- numpy reference (per-core) ----------------

def ref_numpy(x: np.ndarray, kernel: np.ndarray) -> np.ndarray:
    b = x.shape[0]
    hat = (x @ kernel).reshape(b, NCAP, DCAP)
    logits = np.zeros((b, NCAP, 1), dtype=x.dtype)
    out = None
    for _ in range(3):
        ex = np.exp(logits - logits.max(axis=1, keepdims=True))
        c = ex / ex.sum(axis=1, keepdims=True)
        s = (c * hat).sum(axis=1, keepdims=True)
        s2 = np.square(s).sum(axis=-1, keepdims=True)
        out = s2 / (1.0 + s2) / np.sqrt(s2 + EPS) * s
        logits = logits + np.einsum("bnd,bd->bn", hat, out[:, 0, :])[:, :, None]
    return out[:, 0, :]


# ---------------- public entry point ----------------

_CACHE = {}

BEST = Cfg()


def prep_core_inputs(x: np.ndarray, kern: np.ndarray, cfg: Cfg):
    """Full inputs -> list of per-core input maps."""
    kpad = make_kpad(np.asarray(kern, dtype=np.float32))
    ident = np.eye(128, dtype=np.float32)
    seed = np.zeros((128, DCAP), dtype=np.float32)
    bc = cfg.bc
    return [
        {"xin": x[i * bc:(i + 1) * bc], "kpad": kpad, "ident": ident,
         "seed": seed}
        for i in range(cfg.n_cores)
    ]


def kernel(inputs: np.ndarray, kernel: np.ndarray) -> np.ndarray:
    """CapsuleLayer forward: inputs [16384, 2048] f32, kernel [2048, 160] f32
    -> [16384, 16] f32. Runs SPMD across 8 NeuronCores (batch split 8 ways)."""
    from concourse.bass_utils import run_bass_kernel_spmd

    cfg = BEST
    assert inputs.shape == (cfg.bc * cfg.n_cores, D)
    assert kernel.shape == (D, NCOL)
    if "nc" not in _CACHE:
        _CACHE["nc"] = build(cfg)
    nc = _CACHE["nc"]

    x = np.ascontiguousarray(inputs, dtype=np.float32)
    in_maps = prep_core_inputs(x, kernel, cfg)
    res = run_bass_kernel_spmd(nc, in_maps, list(range(cfg.n_cores)))
    return np.concatenate(
        [res.results[i]["yout"] for i in range(cfg.n_cores)], axis=0)


# revision 11
# speedup vs baseline: 1.1036x; 1.1036x over previous
"""CapsuleLayer Bass/Tile kernel for TRN2 (one NeuronCore; replicated SPMD x8).

Per core: xin [Bc, 2048] f32, kpad [2048, 176] f16 (kernel cols 0:160,
col 160:176 = 0.1*sum of capsule blocks), ident [128, 128] f32r.
Output yout [Bc, 16] f32.

Pipeline per 128-sample b-tile:
  DMA xin tile -> SBUF (natural layout, f32)
  PE transpose 16x [128,128] -> 2x PSUM [128,1024] (f32r pass-through)
  ACT/DVE copy PSUM -> SBUF f16 (transposed tiles = matmul lhsT)
  PE matmul (data-as-weights) x16 accumulating f16 kpad-streams
    -> PSUM hat [128, 176] f32
  copy hat[:, :160] -> f32 wide staging, [160:176] -> f32 s1 staging
Routing per group of G b-tiles on wide [128, G*160] f32 layout.
"""

from dataclasses import dataclass

import numpy as np

import concourse.bacc as bacc
import concourse.tile as tile
from concourse import mybir

NCAP = 10
DCAP = 16
EPS = 1e-7
D = 2048
NCOL = NCAP * DCAP  # 160
S1COL = NCOL + DCAP  # 176


@dataclass
class Cfg:
    n_btiles: int = 16          # 128-sample tiles per core
    group_sizes: str = "10,4,2"
    n_cores: int = 8
    reps: int = 1
    dma_btiles: int = 1         # b-tiles per input DMA (1/2/4/8)
    ablate: str = "full"        # full | noroute
    loop_reps: int = 0          # >0: wrap body in a hardware For_i loop
    stage_eng: str = "AD"       # engines for the 2 half stage copies
    hat_eng: str = "A"          # engine for hat [160] f32 copy
    s1_eng: str = "A"           # engine for s1 [16] f32 copy
    wsum_mul_eng: str = "D"     # engine for wsum big muls
    dots_mul_eng: str = "D"     # engine for dots big muls
    use_divide: bool = False    # DVE divide op in squash (no ISA support)
    hoist_kpad: bool = True     # load kpad outside the loop body
    nat_bufs: int = 6
    tt_bufs: int = 6
    pstage_bufs: int = 3
    phat_bufs: int = 2

    @property
    def bc(self):
        return self.n_btiles * 128


def make_kpad(kernel: np.ndarray) -> np.ndarray:
    """[2048, 160] f32 -> [2048, 176] f16 with col 160:176 = 0.1 * capsule sum."""
    d, ncol = kernel.shape
    assert (d, ncol) == (D, NCOL)
    kpad = np.zeros((d, S1COL), dtype=np.float64)
    kpad[:, :NCOL] = kernel
    kpad[:, NCOL:S1COL] = 0.1 * kernel.astype(np.float64).reshape(
        d, NCAP, DCAP).sum(axis=1)
    return kpad.astype(np.float16)


def build(cfg: Cfg):
    nc = bacc.Bacc("TRN2", target_bir_lowering=False, debug=False,
                   num_devices=cfg.n_cores)
    f32 = mybir.dt.float32
    f32r = mybir.dt.float32r
    f16 = mybir.dt.float16

    NB = cfg.n_btiles
    sizes = [int(s) for s in cfg.group_sizes.split(",")]
    assert sum(sizes) == NB

    def eng(c):
        return {"A": nc.scalar, "D": nc.vector, "P": nc.gpsimd}[c]

    def copy_with(c, out, in_):
        if c == "A":
            nc.scalar.copy(out, in_)
        elif c == "D":
            nc.vector.tensor_copy(out, in_)
        else:
            nc.gpsimd.tensor_copy(out, in_)

    eps_t = nc.alloc_sbuf_tensor("const-eps", [128, 1], f32)
    nc.gpsimd.memset(eps_t.ap(), EPS)
    nc.const_aps.aps[(f32, EPS)] = eps_t.ap()
    nc.all_engine_barrier()

    xin = nc.dram_tensor("xin", [cfg.bc, D], f32r, kind="ExternalInput")
    kpad = nc.dram_tensor("kpad", [D, S1COL], f16, kind="ExternalInput")
    ident = nc.dram_tensor("ident", [128, 128], f32r, kind="ExternalInput")
    seed = nc.dram_tensor("seed", [128, DCAP], f32, kind="ExternalInput")
    yout = nc.dram_tensor("yout", [cfg.bc, DCAP], f32, kind="ExternalOutput")

    with tile.TileContext(nc) as tc:
        with (
            tc.tile_pool(name="const", bufs=1) as constp,
            tc.tile_pool(name="nat", bufs=cfg.nat_bufs) as natp,
            tc.tile_pool(name="tT", bufs=cfg.tt_bufs) as tTp,
            tc.tile_pool(name="pstage", bufs=cfg.pstage_bufs, space="PSUM") as pstagep,
            tc.tile_pool(name="phat", bufs=cfg.phat_bufs, space="PSUM") as phatp,
            tc.tile_pool(name="hatw", bufs=2) as hatwp,
            tc.tile_pool(name="rt", bufs=2) as rtp,
            tc.tile_pool(name="sm", bufs=2) as smp,
            tc.tile_pool(name="outs", bufs=2) as outsp,
        ):
            id_t = constp.tile([128, 128], f32r, tag="ident")
            nc.sync.dma_start(id_t[:], ident[:, :])
            seed_t = constp.tile([128, DCAP], f32, tag="seed")
            nc.sync.dma_start(seed_t[:], seed[:, :])
            kp_t = constp.tile([128, 16 * S1COL], f16, tag="kpad")

            def load_kpad():
                # kpad[j*128 + p, c] -> kp_t[p, j*176 + c]
                nc.sync.dma_start(
                    kp_t[:].rearrange("p (j c) -> p j c", j=16),
                    kpad[:, :].rearrange("(j p) c -> p j c", p=128),
                )

            xv = xin[:, :].rearrange("(t p) d -> t p d", p=128)

            DB = cfg.dma_btiles
            nat_slices = {}  # i -> (tile, col offset)
            kpad_loaded = [False]

            def load_chunk(i0):
                nat = natp.tile([128, DB * D], f32r, tag="nat")
                if DB == 1:
                    nc.sync.dma_start(nat[:], xv[i0])
                else:
                    nc.sync.dma_start(
                        nat[:].rearrange("p (t d) -> p t d", t=DB),
                        xin[:, :].rearrange("(c t p) d -> c p t d",
                                            t=DB, p=128)[i0 // DB],
                    )
                for t in range(DB):
                    nat_slices[i0 + t] = (nat, t * D)
                if not kpad_loaded[0] and not cfg.hoist_kpad:
                    kpad_loaded[0] = True
                    load_kpad()

            def run_group(i0, G, gi):
                yv = yout[i0 * 128:(i0 + G) * 128, :].rearrange(
                    "(g p) d -> p g d", p=128)
                hatw = hatwp.tile([128, G * NCOL], f32, tag="hatw")
                s1w = hatwp.tile([128, G * DCAP], f32, tag="s1w")
                for g in range(G):
                    i = i0 + g
                    if i % DB == 0:
                        load_chunk(i)
                    nat, off = nat_slices.pop(i)
                    tts = []
                    for h in range(2):
                        ps = pstagep.tile([128, 1024], f32r, tag="pstage")
                        for jj in range(8):
                            j = h * 8 + jj
                            nc.tensor.matmul(
                                ps[:, jj * 128:(jj + 1) * 128],
                                nat[:, off + j * 128:off + (j + 1) * 128],
                                id_t[:],
                                is_transpose=True,
                                start=(jj % 4 == 0),
                                stop=(jj % 4 == 3),
                            )
                        tt = tTp.tile([128, 1024], f16, tag="tT")
                        copy_with(cfg.stage_eng[h], tt[:], ps[:])
                        tts.append(tt)
                    ph = phatp.tile([128, S1COL], f32, tag="phat")
                    for j in range(16):
                        h, jj = divmod(j, 8)
                        nc.tensor.matmul(
                            ph[:],
                            tts[h][:, jj * 128:(jj + 1) * 128],
                            kp_t[:, j * S1COL:(j + 1) * S1COL],
                            start=(j == 0),
                            stop=(j == 15),
                        )
                    copy_with(cfg.hat_eng,
                              hatw[:, g * NCOL:(g + 1) * NCOL], ph[:, :NCOL])
                    copy_with(cfg.s1_eng,
                              s1w[:, g * DCAP:(g + 1) * DCAP],
                              ph[:, NCOL:S1COL])
                if cfg.ablate == "noroute":
                    nc.sync.dma_start(
                        yv,
                        s1w[:].rearrange("p (g q) -> p g q", g=G))
                    return

                # ---- routing on [128, G*160] bf16 ----
                H = hatw[:]
                Hgnd = H.rearrange("p (g n d) -> p g n d", g=G, n=NCAP)
                wse = eng(cfg.wsum_mul_eng)
                dse = eng(cfg.dots_mul_eng)

                def squash_comb(su, r, tag):
                    """combined scale c s.t. v = c * su, where s = su * r
                    (r None -> s = su). Returns [128, G] AP."""
                    sq = smp.tile([128, G * DCAP], f32, tag=f"sq{tag}")
                    nc.vector.tensor_mul(sq[:], su, su)
                    m2 = smp.tile([128, G], f32, tag=f"m2{tag}")
                    nc.vector.tensor_reduce(
                        m2[:], sq[:].rearrange("p (g d) -> p g d", g=G),
                        axis=mybir.AxisListType.X, op=mybir.AluOpType.add)
                    if r is not None:
                        rr = smp.tile([128, G], f32, tag=f"rr{tag}")
                        nc.vector.tensor_mul(rr[:], r, r)
                        n2 = smp.tile([128, G], f32, tag=f"n2{tag}")
                        nc.vector.tensor_mul(n2[:], m2[:], rr[:])
                    else:
                        n2 = m2
                    sr = smp.tile([128, G], f32, tag=f"sr{tag}")
                    nc.scalar.activation(sr[:], n2[:],
                                         mybir.ActivationFunctionType.Sqrt,
                                         bias=EPS)
                    den = smp.tile([128, G], f32, tag=f"den{tag}")
                    nc.vector.scalar_tensor_tensor(
                        den[:], n2[:], 1.0, sr[:],
                        op0=mybir.AluOpType.add, op1=mybir.AluOpType.mult)
                    sc = smp.tile([128, G], f32, tag=f"sc{tag}")
                    if cfg.use_divide:
                        nc.vector.tensor_tensor(
                            sc[:], n2[:], den[:], op=mybir.AluOpType.divide)
                    else:
                        rec = smp.tile([128, G], f32, tag=f"rec{tag}")
                        nc.vector.reciprocal(rec[:], den[:])
                        nc.vector.tensor_mul(sc[:], n2[:], rec[:])
                    if r is not None:
                        comb = smp.tile([128, G], f32, tag=f"comb{tag}")
                        nc.vector.tensor_mul(comb[:], sc[:], r)
                        return comb
                    return sc

                def dots_d(src_gd, tag):
                    """r[g,n] = sum_d H[g,n,d] * src[g,d] -> [128, G*NCAP] f32"""
                    tmp = rtp.tile([128, G * NCOL], f32, tag=f"dt{tag}")
                    bc = (src_gd.rearrange("p (g d) -> p g d", g=G)
                          .unsqueeze(2).broadcast_to((128, G, NCAP, DCAP)))
                    dse.tensor_mul(
                        tmp[:].rearrange("p (g n d) -> p g n d", g=G, n=NCAP),
                        Hgnd, bc)
                    out = rtp.tile([128, G * NCAP], f32, tag=f"dr{tag}")
                    nc.vector.tensor_reduce(
                        out[:], tmp[:].rearrange("p (g n d) -> p g n d",
                                                 g=G, n=NCAP),
                        axis=mybir.AxisListType.X, op=mybir.AluOpType.add)
                    return out

                def wsum_n(e_gn, tag):
                    """su[g,d] = sum_n H[g,n,d] * e[g,n] -> [128, G*DCAP] f32"""
                    tmp = rtp.tile([128, G * NCOL], f32, tag=f"wt{tag}")
                    bc = e_gn.unsqueeze(3).broadcast_to((128, G, NCAP, DCAP))
                    wse.tensor_mul(
                        tmp[:].rearrange("p (g n d) -> p g n d", g=G, n=NCAP),
                        Hgnd, bc)
                    out = rtp.tile([128, G * DCAP], f32, tag=f"ws{tag}")
                    nc.vector.tensor_reduce(
                        out[:], tmp[:].rearrange("p (g n d) -> p g d n",
                                                 g=G, n=NCAP),
                        axis=mybir.AxisListType.X, op=mybir.AluOpType.add)
                    return out

                def softmax_recip(t_gn, tag):
                    """e = exp(t) [128, G*NCAP] f32; r = 1/sum_n e [128, G]"""
                    e = rtp.tile([128, G * NCAP], f32, tag=f"e{tag}")
                    nc.scalar.activation(e[:], t_gn,
                                         mybir.ActivationFunctionType.Exp)
                    se = smp.tile([128, G], f32, tag=f"se{tag}")
                    nc.vector.tensor_reduce(
                        se[:], e[:].rearrange("p (g n) -> p g n", g=G),
                        axis=mybir.AxisListType.X, op=mybir.AluOpType.add)
                    ri = smp.tile([128, G], f32, tag=f"ri{tag}")
                    nc.vector.reciprocal(ri[:], se[:])
                    return e, ri

                gv = lambda ap: ap.rearrange("p (g d) -> p g d", g=G)
                nv = lambda ap: ap.rearrange("p (g n) -> p g n", g=G)

                # iter 1: s1 (pre-scaled mean) came from the matmul
                comb1 = squash_comb(s1w[:], None, "1")  # v1 = comb1*s1
                r2 = dots_d(s1w[:], "2")                # u.s1
                t2 = rtp.tile([128, G * NCAP], f32, tag="t2")
                nc.vector.tensor_mul(
                    nv(t2[:]), nv(r2[:]),
                    comb1[:].unsqueeze(2).broadcast_to((128, G, NCAP)))

                # iter 2
                e2, r2i = softmax_recip(t2[:], "2")
                s2u = wsum_n(nv(e2[:]), "2")
                comb2 = squash_comb(s2u[:], r2i[:], "2")  # v2 = comb2*s2u
                r3 = dots_d(s2u[:], "3")                  # u.s2u
                t3 = rtp.tile([128, G * NCAP], f32, tag="t3")
                nc.vector.tensor_mul(
                    nv(t3[:]), nv(r3[:]),
                    comb2[:].unsqueeze(2).broadcast_to((128, G, NCAP)))
                nc.vector.tensor_add(t3[:], t3[:], t2[:])

                # iter 3
                e3, r3i = softmax_recip(t3[:], "3")
                s3u = wsum_n(nv(e3[:]), "3")
                comb3 = squash_comb(s3u[:], r3i[:], "3")
                v3 = outsp.tile([128, G * DCAP], f32, tag="v3")
                nc.vector.tensor_mul(
                    gv(v3[:]), gv(s3u[:]),
                    comb3[:].unsqueeze(2).broadcast_to((128, G, DCAP)))
                nc.sync.dma_start(
                    yv,
                    v3[:].rearrange("p (g d) -> p g d", g=G))

            def run_all():
                i0 = 0
                for gi, G in enumerate(sizes):
                    run_group(i0, G, gi)
                    i0 += G

            if cfg.hoist_kpad:
                load_kpad()
            if cfg.loop_reps > 0:
                with tc.For_i(0, cfg.loop_reps, 1,
                              hint_engines=(mybir.EngineType.PE,)):
                    run_all()
            else:
                for _rep in range(cfg.reps):
                    run_all()

    nc.compile()
    return nc


# ---------------- numpy reference (per-core) ----------------

def ref_numpy(x: np.ndarray, kernel: np.ndarray) -> np.ndarray:
    b = x.shape[0]
    hat = (x @ kernel).reshape(b, NCAP, DCAP)
    logits = np.zeros((b, NCAP, 1), dtype=x.dtype)
    out = None
    for _ in range(3):
        ex = np.exp(logits - logits.max(axis=1, keepdims=True))
        c = ex / ex.sum(axis=1, keepdims=True)
        s = (c * hat).sum(axis=1, keepdims=True)
        s2 = np.square(s).sum(axis=-1, keepdims=True)
        out = s2 / (1.0 + s2) / np.sqrt(s2 + EPS) * s
        logits = logits + np.einsum("bnd,bd->bn", hat, out[:, 0, :])[:, :, None]
    return out[:, 0, :]


# ---------------- public entry point ----------------

_CACHE = {}

BEST = Cfg()


def prep_core_inputs(x: np.ndarray, kern: np.ndarray, cfg: Cfg):
    """Full inputs -> list of per-core input maps."""
    kpad = make_kpad(np.asarray(kern, dtype=np.float32))
    ident = np.eye(128, dtype=np.float32)
    seed = np.zeros((128, DCAP), dtype=np.float32)
    bc = cfg.bc
    return [
        {"xin": x[i * bc:(i + 1) * bc], "kpad": kpad, "ident": ident,
         "seed": seed}
        for i in range(cfg.n_cores)
    ]


def kernel(inputs: np.ndarray, kernel: np.ndarray) -> np.ndarray:
    """CapsuleLayer forward: inputs [16384, 2048] f32, kernel [2048, 160] f32
    -> [16384, 16] f32. Runs SPMD across 8 NeuronCores (batch split 8 ways)."""
    from concourse.bass_utils import run_bass_kernel_spmd

    cfg = BEST
    assert inputs.shape == (cfg.bc * cfg.n_cores, D)
    assert kernel.shape == (D, NCOL)
    if "nc" not in _CACHE:
        _CACHE["nc"] = build(cfg)
    nc = _CACHE["nc"]

    x = np.ascontiguousarray(inputs, dtype=np.float32)
    in_maps = prep_core_inputs(x, kernel, cfg)
    res = run_bass_kernel_spmd(nc, in_maps, list(range(cfg.n_cores)))
    return np.concatenate(
        [res.results[i]["yout"] for i in range(cfg.n_cores)], axis=0)


# revision 14
# speedup vs baseline: 1.1516x; 1.0435x over previous
"""CapsuleLayer Bass/Tile kernel for TRN2 (one NeuronCore; replicated SPMD x8).

Per core: xin [Bc, 2048] f32, kpad [2048, 176] f16 (kernel cols 0:160,
col 160:176 = 0.1*sum of capsule blocks), ident [128, 128] f32r.
Output yout [Bc, 16] f32.

Pipeline per 128-sample b-tile:
  DMA xin tile -> SBUF (natural layout, f32)
  PE transpose 16x [128,128] -> 2x PSUM [128,1024] (f32r pass-through)
  ACT/DVE copy PSUM -> SBUF f16 (transposed tiles = matmul lhsT)
  PE matmul (data-as-weights) x16 accumulating f16 kpad-streams
    -> PSUM hat [128, 176] f32
  copy hat[:, :160] -> f32 wide staging, [160:176] -> f32 s1 staging
Routing per group of G b-tiles on wide [128, G*160] f32 layout.
"""

from dataclasses import dataclass

import numpy as np

import concourse.bacc as bacc
import concourse.tile as tile
from concourse import mybir

NCAP = 10
DCAP = 16
EPS = 1e-7
D = 2048
NCOL = NCAP * DCAP  # 160
S1COL = NCOL + DCAP  # 176


@dataclass
class Cfg:
    n_btiles: int = 16          # 128-sample tiles per core
    group_sizes: str = "10,4,2"
    n_cores: int = 8
    reps: int = 1
    dma_btiles: int = 1         # b-tiles per input DMA (1/2/4/8)
    ablate: str = "full"        # full | noroute
    loop_reps: int = 0          # >0: wrap body in a hardware For_i loop
    stage_eng: str = "AD"       # engines for the 2 half stage copies
    hat_eng: str = "A"          # engine for hat [160] f32 copy
    s1_eng: str = "A"           # engine for s1 [16] f32 copy
    wsum_mul_eng: str = "D"     # engine for wsum big muls
    dots_mul_eng: str = "D"     # engine for dots big muls
    use_divide: bool = False    # DVE divide op in squash (no ISA support)
    hoist_kpad: bool = True     # load kpad outside the loop body
    dma_stage: bool = False     # dead: DMA cannot read PSUM
    nat_bufs: int = 6
    tt_bufs: int = 6
    pstage_bufs: int = 3
    phat_bufs: int = 2

    @property
    def bc(self):
        return self.n_btiles * 128


def make_kpad(kernel: np.ndarray) -> np.ndarray:
    """[2048, 160] f32 -> [2048, 176] f16 with col 160:176 = 0.1 * capsule sum."""
    d, ncol = kernel.shape
    assert (d, ncol) == (D, NCOL)
    kpad = np.zeros((d, S1COL), dtype=np.float64)
    kpad[:, :NCOL] = kernel
    kpad[:, NCOL:S1COL] = 0.1 * kernel.astype(np.float64).reshape(
        d, NCAP, DCAP).sum(axis=1)
    return kpad.astype(np.float16)


def build(cfg: Cfg):
    nc = bacc.Bacc("TRN2", target_bir_lowering=False, debug=False,
                   num_devices=cfg.n_cores)
    f32 = mybir.dt.float32
    f32r = mybir.dt.float32r
    f16 = mybir.dt.float16

    NB = cfg.n_btiles
    sizes = [int(s) for s in cfg.group_sizes.split(",")]
    assert sum(sizes) == NB

    def eng(c):
        return {"A": nc.scalar, "D": nc.vector, "P": nc.gpsimd}[c]

    def copy_with(c, out, in_):
        if c == "A":
            nc.scalar.copy(out, in_)
        elif c == "D":
            nc.vector.tensor_copy(out, in_)
        else:
            nc.gpsimd.tensor_copy(out, in_)

    eps_t = nc.alloc_sbuf_tensor("const-eps", [128, 1], f32)
    nc.gpsimd.memset(eps_t.ap(), EPS)
    nc.const_aps.aps[(f32, EPS)] = eps_t.ap()
    nc.all_engine_barrier()

    xin = nc.dram_tensor("xin", [cfg.bc, D], f32r, kind="ExternalInput")
    kpad = nc.dram_tensor("kpad", [D, S1COL], f16, kind="ExternalInput")
    ident = nc.dram_tensor("ident", [128, 128], f32r, kind="ExternalInput")
    seed = nc.dram_tensor("seed", [128, DCAP], f32, kind="ExternalInput")
    yout = nc.dram_tensor("yout", [cfg.bc, DCAP], f32, kind="ExternalOutput")

    with tile.TileContext(nc) as tc:
        with (
            tc.tile_pool(name="const", bufs=1) as constp,
            tc.tile_pool(name="nat", bufs=cfg.nat_bufs) as natp,
            tc.tile_pool(name="tT", bufs=cfg.tt_bufs) as tTp,
            tc.tile_pool(name="pstage", bufs=cfg.pstage_bufs, space="PSUM") as pstagep,
            tc.tile_pool(name="phat", bufs=cfg.phat_bufs, space="PSUM") as phatp,
            tc.tile_pool(name="hatw", bufs=2) as hatwp,
            tc.tile_pool(name="rt", bufs=2) as rtp,
            tc.tile_pool(name="sm", bufs=2) as smp,
            tc.tile_pool(name="outs", bufs=2) as outsp,
        ):
            id_t = constp.tile([128, 128], f32r, tag="ident")
            nc.sync.dma_start(id_t[:], ident[:, :])
            seed_t = constp.tile([128, DCAP], f32, tag="seed")
            nc.sync.dma_start(seed_t[:], seed[:, :])
            kp_t = constp.tile([128, 16 * S1COL], f16, tag="kpad")
            kp32 = None
            if cfg.dma_stage:
                kp32 = constp.tile([128, 8 * 256], f32r, tag="kpad32")
                nc.gpsimd.memset(kp32[:], 0.0)

            def load_kpad():
                # kpad[j*128 + p, c] -> kp_t[p, j*176 + c]
                nc.sync.dma_start(
                    kp_t[:].rearrange("p (j c) -> p j c", j=16),
                    kpad[:, :].rearrange("(j p) c -> p j c", p=128),
                )
                if cfg.dma_stage:
                    # chunks 8..15 as f32r (cast once from f16)
                    nc.vector.tensor_copy(
                        kp32[:].rearrange("p (j c) -> p j c", j=8)[:, :, :S1COL],
                        kp_t[:].rearrange("p (j c) -> p j c", j=16)[:, 8:, :],
                    )

            xv = xin[:, :].rearrange("(t p) d -> t p d", p=128)

            DB = cfg.dma_btiles
            nat_slices = {}  # i -> (tile, col offset)
            kpad_loaded = [False]

            def load_chunk(i0):
                nat = natp.tile([128, DB * D], f32r, tag="nat")
                if DB == 1:
                    nc.sync.dma_start(nat[:], xv[i0])
                else:
                    nc.sync.dma_start(
                        nat[:].rearrange("p (t d) -> p t d", t=DB),
                        xin[:, :].rearrange("(c t p) d -> c p t d",
                                            t=DB, p=128)[i0 // DB],
                    )
                for t in range(DB):
                    nat_slices[i0 + t] = (nat, t * D)
                if not kpad_loaded[0] and not cfg.hoist_kpad:
                    kpad_loaded[0] = True
                    load_kpad()

            def run_group(i0, G, gi):
                yv = yout[i0 * 128:(i0 + G) * 128, :].rearrange(
                    "(g p) d -> p g d", p=128)
                hatw = hatwp.tile([128, G * NCOL], f32, tag="hatw")
                s1w = hatwp.tile([128, G * DCAP], f32, tag="s1w")
                for g in range(G):
                    i = i0 + g
                    if i % DB == 0:
                        load_chunk(i)
                    nat, off = nat_slices.pop(i)
                    tts = []
                    for h in range(2):
                        ps = pstagep.tile([128, 1024], f32r, tag="pstage")
                        for jj in range(8):
                            j = h * 8 + jj
                            nc.tensor.matmul(
                                ps[:, jj * 128:(jj + 1) * 128],
                                nat[:, off + j * 128:off + (j + 1) * 128],
                                id_t[:],
                                is_transpose=True,
                                start=(jj % 4 == 0),
                                stop=(jj % 4 == 3),
                            )
                        if cfg.dma_stage and h == 1:
                            tt = tTp.tile([128, 1024], f32r, tag="tT32")
                            nc.scalar.dma_start(tt[:], ps[:])
                        else:
                            tt = tTp.tile([128, 1024], f16, tag="tT")
                            copy_with(cfg.stage_eng[h], tt[:], ps[:])
                        tts.append(tt)
                    ph = phatp.tile([128, 256 if cfg.dma_stage else S1COL],
                                    f32, tag="phat")
                    for j in range(16):
                        h, jj = divmod(j, 8)
                        if cfg.dma_stage and h == 1:
                            rhs = kp32[:, jj * 256:jj * 256 + 256]
                        else:
                            rhs = kp_t[:, j * S1COL:(j + 1) * S1COL]
                        nc.tensor.matmul(
                            ph[:, :256] if (cfg.dma_stage and h == 1)
                            else ph[:, :S1COL],
                            tts[h][:, jj * 128:(jj + 1) * 128],
                            rhs,
                            start=(j == 0),
                            stop=(j == 15),
                        )
                    copy_with(cfg.hat_eng,
                              hatw[:, g * NCOL:(g + 1) * NCOL], ph[:, :NCOL])
                    copy_with(cfg.s1_eng,
                              s1w[:, g * DCAP:(g + 1) * DCAP],
                              ph[:, NCOL:S1COL])
                if cfg.ablate == "noroute":
                    nc.sync.dma_start(
                        yv,
                        s1w[:].rearrange("p (g q) -> p g q", g=G))
                    return

                # ---- routing on [128, G*160] bf16 ----
                H = hatw[:]
                Hgnd = H.rearrange("p (g n d) -> p g n d", g=G, n=NCAP)
                wse = eng(cfg.wsum_mul_eng)
                dse = eng(cfg.dots_mul_eng)

                def squash_comb(su, r, tag):
                    """combined scale c s.t. v = c * su, where s = su * r
                    (r None -> s = su). Returns [128, G] AP."""
                    sq = smp.tile([128, G * DCAP], f32, tag=f"sq{tag}")
                    nc.vector.tensor_mul(sq[:], su, su)
                    m2 = smp.tile([128, G], f32, tag=f"m2{tag}")
                    nc.vector.tensor_reduce(
                        m2[:], sq[:].rearrange("p (g d) -> p g d", g=G),
                        axis=mybir.AxisListType.X, op=mybir.AluOpType.add)
                    if r is not None:
                        rr = smp.tile([128, G], f32, tag=f"rr{tag}")
                        nc.vector.tensor_mul(rr[:], r, r)
                        n2 = smp.tile([128, G], f32, tag=f"n2{tag}")
                        nc.vector.tensor_mul(n2[:], m2[:], rr[:])
                    else:
                        n2 = m2
                    sr = smp.tile([128, G], f32, tag=f"sr{tag}")
                    nc.scalar.activation(sr[:], n2[:],
                                         mybir.ActivationFunctionType.Sqrt,
                                         bias=EPS)
                    den = smp.tile([128, G], f32, tag=f"den{tag}")
                    nc.vector.scalar_tensor_tensor(
                        den[:], n2[:], 1.0, sr[:],
                        op0=mybir.AluOpType.add, op1=mybir.AluOpType.mult)
                    sc = smp.tile([128, G], f32, tag=f"sc{tag}")
                    if cfg.use_divide:
                        nc.vector.tensor_tensor(
                            sc[:], n2[:], den[:], op=mybir.AluOpType.divide)
                    else:
                        rec = smp.tile([128, G], f32, tag=f"rec{tag}")
                        nc.vector.reciprocal(rec[:], den[:])
                        nc.vector.tensor_mul(sc[:], n2[:], rec[:])
                    if r is not None:
                        comb = smp.tile([128, G], f32, tag=f"comb{tag}")
                        nc.vector.tensor_mul(comb[:], sc[:], r)
                        return comb
                    return sc

                def dots_d(src_gd, tag):
                    """r[g,n] = sum_d H[g,n,d] * src[g,d] -> [128, G*NCAP] f32"""
                    tmp = rtp.tile([128, G * NCOL], f32, tag=f"dt{tag}")
                    bc = (src_gd.rearrange("p (g d) -> p g d", g=G)
                          .unsqueeze(2).broadcast_to((128, G, NCAP, DCAP)))
                    dse.tensor_mul(
                        tmp[:].rearrange("p (g n d) -> p g n d", g=G, n=NCAP),
                        Hgnd, bc)
                    out = rtp.tile([128, G * NCAP], f32, tag=f"dr{tag}")
                    nc.vector.tensor_reduce(
                        out[:], tmp[:].rearrange("p (g n d) -> p g n d",
                                                 g=G, n=NCAP),
                        axis=mybir.AxisListType.X, op=mybir.AluOpType.add)
                    return out

                def wsum_n(e_gn, tag):
                    """su[g,d] = sum_n H[g,n,d] * e[g,n] -> [128, G*DCAP] f32"""
                    tmp = rtp.tile([128, G * NCOL], f32, tag=f"wt{tag}")
                    bc = e_gn.unsqueeze(3).broadcast_to((128, G, NCAP, DCAP))
                    wse.tensor_mul(
                        tmp[:].rearrange("p (g n d) -> p g n d", g=G, n=NCAP),
                        Hgnd, bc)
                    out = rtp.tile([128, G * DCAP], f32, tag=f"ws{tag}")
                    nc.vector.tensor_reduce(
                        out[:], tmp[:].rearrange("p (g n d) -> p g d n",
                                                 g=G, n=NCAP),
                        axis=mybir.AxisListType.X, op=mybir.AluOpType.add)
                    return out

                def softmax_recip(t_gn, tag):
                    """e = exp(t) [128, G*NCAP] f32; r = 1/sum_n e [128, G]"""
                    e = rtp.tile([128, G * NCAP], f32, tag=f"e{tag}")
                    nc.scalar.activation(e[:], t_gn,
                                         mybir.ActivationFunctionType.Exp)
                    se = smp.tile([128, G], f32, tag=f"se{tag}")
                    nc.vector.tensor_reduce(
                        se[:], e[:].rearrange("p (g n) -> p g n", g=G),
                        axis=mybir.AxisListType.X, op=mybir.AluOpType.add)
                    ri = smp.tile([128, G], f32, tag=f"ri{tag}")
                    nc.vector.reciprocal(ri[:], se[:])
                    return e, ri

                gv = lambda ap: ap.rearrange("p (g d) -> p g d", g=G)
                nv = lambda ap: ap.rearrange("p (g n) -> p g n", g=G)

                # iter 1: s1 (pre-scaled mean) came from the matmul
                comb1 = squash_comb(s1w[:], None, "1")  # v1 = comb1*s1
                r2 = dots_d(s1w[:], "2")                # u.s1
                t2 = rtp.tile([128, G * NCAP], f32, tag="t2")
                nc.vector.tensor_mul(
                    nv(t2[:]), nv(r2[:]),
                    comb1[:].unsqueeze(2).broadcast_to((128, G, NCAP)))

                # iter 2
                e2, r2i = softmax_recip(t2[:], "2")
                s2u = wsum_n(nv(e2[:]), "2")
                comb2 = squash_comb(s2u[:], r2i[:], "2")  # v2 = comb2*s2u
                r3 = dots_d(s2u[:], "3")                  # u.s2u
                t3 = rtp.tile([128, G * NCAP], f32, tag="t3")
                nc.vector.tensor_mul(
                    nv(t3[:]), nv(r3[:]),
                    comb2[:].unsqueeze(2).broadcast_to((128, G, NCAP)))
                nc.vector.tensor_add(t3[:], t3[:], t2[:])

                # iter 3
                e3, r3i = softmax_recip(t3[:], "3")
                s3u = wsum_n(nv(e3[:]), "3")
                comb3 = squash_comb(s3u[:], r3i[:], "3")
                v3 = outsp.tile([128, G * DCAP], f32, tag="v3")
                nc.vector.tensor_mul(
                    gv(v3[:]), gv(s3u[:]),
                    comb3[:].unsqueeze(2).broadcast_to((128, G, DCAP)))
                nc.sync.dma_start(
                    yv,
                    v3[:].rearrange("p (g d) -> p g d", g=G))

            def run_all():
                i0 = 0
                for gi, G in enumerate(sizes):
                    run_group(i0, G, gi)
                    i0 += G

            if cfg.hoist_kpad:
                load_kpad()
            if cfg.loop_reps > 0:
                with tc.For_i(0, cfg.loop_reps, 1,
                              hint_engines=(mybir.EngineType.PE,)):
                    run_all()
            else:
                for _rep in range(cfg.reps):
                    run_all()

    nc.compile()
    return nc


# ---------------- numpy reference (per-core) ----------------

def ref_numpy(x: np.ndarray, kernel: np.ndarray) -> np.ndarray:
    b = x.shape[0]
    hat = (x @ kernel).reshape(b, NCAP, DCAP)
    logits = np.zeros((b, NCAP, 1), dtype=x.dtype)
    out = None
    for _ in range(3):
        ex = np.exp(logits - logits.max(axis=1, keepdims=True))
        c = ex / ex.sum(axis=1, keepdims=True)
        s = (c * hat).sum(axis=1, keepdims=True)
        s2 = np.square(s).sum(axis=-1, keepdims=True)
        out = s2 / (1.0 + s2) / np.sqrt(s2 + EPS) * s
        logits = logits + np.einsum("bnd,bd->bn", hat, out[:, 0, :])[:, :, None]
    return out[:, 0, :]


# ---------------- public entry point ----------------

_CACHE = {}

BEST = Cfg()


def prep_core_inputs(x: np.ndarray, kern: np.ndarray, cfg: Cfg):
    """Full inputs -> list of per-core input maps."""
    kpad = make_kpad(np.asarray(kern, dtype=np.float32))
    ident = np.eye(128, dtype=np.float32)
    seed = np.zeros((128, DCAP), dtype=np.float32)
    bc = cfg.bc
    return [
        {"xin": x[i * bc:(i + 1) * bc], "kpad": kpad, "ident": ident,
         "seed": seed}
        for i in range(cfg.n_cores)
    ]


def kernel(inputs: np.ndarray, kernel: np.ndarray) -> np.ndarray:
    """CapsuleLayer forward: inputs [16384, 2048] f32, kernel [2048, 160] f32
    -> [16384, 16] f32. Runs SPMD across 8 NeuronCores (batch split 8 ways)."""
    from concourse.bass_utils import run_bass_kernel_spmd

    cfg = BEST
    assert inputs.shape == (cfg.bc * cfg.n_cores, D)
    assert kernel.shape == (D, NCOL)
    if "nc" not in _CACHE:
        _CACHE["nc"] = build(cfg)
    nc = _CACHE["nc"]

    x = np.ascontiguousarray(inputs, dtype=np.float32)
    in_maps = prep_core_inputs(x, kernel, cfg)
    res = run_bass_kernel_spmd(nc, in_maps, list(range(cfg.n_cores)))
    return np.concatenate(
        [res.results[i]["yout"] for i in range(cfg.n_cores)], axis=0)
